# revision 16
# baseline (speedup 1.0000x reference)
"""Trainium2 Bass kernel for DecoderAttn ('general' attention score + softmax).

Reference computation (per batch b):
    energies[t] = dec[b] . (enc[b,t] @ W.T + bias)
    attn = softmax(energies over t)

Algebraic rewrite used here:
    energies[t] = enc[b,t] . (W.T @ dec[b])  +  (bias . dec[b])
The second term is constant over t, so it drops out of the softmax exactly.
This turns an O(B*T*H^2) matmul chain into an O(B*H^2 + B*T*H) streaming
problem: precompute v[b] = (dec @ W)[b] on the tensor engine, stream the
encoder through one fused DVE op per [128,1024] tile, then a tiny softmax.

Sharding: data-parallel over batch B=32 across 8 NeuronCores (4 batches per
core). W is sharded 8-ways by rows (o-chunks): each core computes a partial
v for all 32 batches over its 128-row W slice, and a ReduceScatter-add hands
core c exactly its own 4 batches' v — cutting per-core DMA from 36.1MB to
~32.7MB vs replicating W.

Performance structure (measured via repeat-slope benching + TimelineSim):
  - fuse: the per-tile dot is ONE DVE scalar_tensor_tensor (out=et*vb,
    accum_out=column of e_t) at ~1.0us/tile — replaces DVE mul + ACT
    accumulate, freeing the ACT engine (~-18us).
    (tensor_tensor_reduce and the gpsimd equivalent crash/fail to compile.)
  - dbuf: the v chain (dec/W load, matmuls, ReduceScatter, broadcast)
    double-buffers its SBUF state across reps, so rep r+1's collective
    overlaps rep r's stream; the RS otherwise costs ~19us/rep (~-14us).
  - staged: the per-batch softmax is split into 5 stages interleaved into
    the NEXT batch's stt stream. The in-order DVE queue otherwise stalls
    ~4us/batch on softmax cross-engine round trips (~-12us).
  - act_copies: startup/softmax PSUM->SBUF copies go to the ACT engine,
    keeping the DVE queue pure stt.
With all of these the steady state is DMA-paced: per-core traffic is
33.55MB enc + 0.5MB W + ~0.2MB dec/cc per workload at the ~358GB/s
per-core HBM ceiling (~95us theoretical floor; DMA-only variant measures
within a few us of the full kernel).
"""

import numpy as np
from contextlib import ExitStack

import concourse.bass as bass
import concourse.tile as tile
from concourse import bacc, mybir, masks
from concourse.bass_utils import run_bass_kernel_spmd

F32 = mybir.dt.float32

B, T, H = 32, 2048, 1024
NCORES = 8
BL = B // NCORES           # batches per core
TCH = T // 128             # 128-row t-chunks per batch
OCH = H // 128             # 128-row o-chunks of W


def build_kernel(bl=BL, t=T, h=H, enc_bufs=24, repeat=1, scr_bufs=4, pair=False,
                 wshard=False, n_cores=NCORES, startup_in_loop=False,
                 startup_dma="scalar", fuse=False, pool_every=0,
                 no_rs=False, dmaonly=False, dbuf=False, act_copies=False,
                 stt2=False, sm2=False, staged=False, pair2=False,
                 sm_bufs=2, eb=2, ps_bufs=3):
    tch = t // 128
    och = h // 128
    nhh = h // 512  # 512-wide halves of the H free dim for matmul N-limit

    nc = bacc.Bacc("TRN2", target_bir_lowering=False, debug=False)

    if wshard:
        # every core gets: dec columns for ITS o-chunk [B, 128], W rows for
        # ITS o-chunk [128, h]; partial v is summed across cores with a
        # ReduceScatter that hands core c exactly its 4 batches' v.
        nb = bl * n_cores
        dec = nc.dram_tensor("dec", [nb, 128], F32, kind="ExternalInput")
        w = nc.dram_tensor("w", [128, h], F32, kind="ExternalInput")
    else:
        dec = nc.dram_tensor("dec", [bl, h], F32, kind="ExternalInput")
        w = nc.dram_tensor("w", [h, h], F32, kind="ExternalInput")
    enc = nc.dram_tensor("enc", [bl, t, h], F32, kind="ExternalInput")
    attn = nc.dram_tensor("attn", [bl, t], F32, kind="ExternalOutput")

    with tile.TileContext(nc) as tc, ExitStack() as ctx:
        const = ctx.enter_context(tc.tile_pool(name="const", bufs=1))
        wpool = ctx.enter_context(tc.tile_pool(name="wpool", bufs=1))
        encp = ctx.enter_context(tc.tile_pool(name="encp", bufs=enc_bufs))
        scr = ctx.enter_context(tc.tile_pool(name="scr", bufs=scr_bufs))
        sm = ctx.enter_context(tc.tile_pool(name="sm", bufs=sm_bufs))
        outp = ctx.enter_context(tc.tile_pool(name="outp", bufs=2))
        psA = ctx.enter_context(tc.tile_pool(name="psA", bufs=2, space="PSUM"))
        psS = ctx.enter_context(tc.tile_pool(name="psS", bufs=ps_bufs, space="PSUM"))

        sdma = getattr(nc, startup_dma)

        # ---- constants ----
        ident = const.tile([128, 128], F32)
        masks.make_identity(nc, ident[:])
        ones = const.tile([1, 128], F32)
        nc.gpsimd.memset(ones[:], 1.0)

        # long-lived state
        epool = ctx.enter_context(tc.tile_pool(name="epool", bufs=eb))
        vrep = 2 if pair else 1
        if not dbuf:
            vb_all = const.tile([128, bl * vrep * h], F32)  # v[b] bcast

        copy_small = nc.scalar.copy if act_copies else nc.vector.tensor_copy

        def do_startup(rep):
            # dbuf alternates the vb buffer across reps so rep r+1's whole
            # v chain (incl. the ReduceScatter) overlaps rep r's enc stream
            if dbuf:
                vb_cur = const.tile([128, bl * vrep * h], F32,
                                    tag=f"vball{rep % 2}")
            else:
                vb_cur = vb_all
            v_sb = const.tile([1, bl * h], F32, tag="v_sb")  # rows on partition 0
            if wshard:
                # phase 1 (sharded W): partial v over this core's o-chunk,
                # ReduceScatter-add across cores
                dec_sb = const.tile([nb, 128], F32, tag="dec_sb")
                decT = const.tile([128, nb], F32, tag="decT")
                pv_sb = const.tile([nb, h], F32, tag="pv_sb")
                cc_in = nc.dram_tensor(f"cc_in{rep}", [nb, h], F32)
                cc_out = nc.dram_tensor(f"cc_out{rep}", [bl, h], F32)

                sdma.dma_start(dec_sb[:], dec[:, :])
                dT_ps = psS.tile([128, nb], F32, tag="small")
                nc.tensor.transpose(dT_ps[:], dec_sb[:, :], ident[0:nb, 0:nb])
                copy_small(decT[:, :], dT_ps[:])
                wt = wpool.tile([128, h], F32, tag="w0")
                sdma.dma_start(wt[:], w[:, :])
                for hh in range(nhh):
                    pv_ps = psA.tile([nb, 512], F32, tag="work")
                    nc.tensor.matmul(
                        pv_ps[:], decT[:, :], wt[:, hh * 512:(hh + 1) * 512],
                        start=True, stop=True,
                    )
                    copy_small(
                        pv_sb[:, hh * 512:(hh + 1) * 512], pv_ps[:]
                    )
                sdma.dma_start(cc_in[:, :], pv_sb[:])
                if no_rs:
                    # timing diagnostic: skip the collective (WRONG results)
                    sdma.dma_start(cc_out[:, :], cc_in[0:bl, :])
                else:
                    nc.gpsimd.collective_compute(
                        "ReduceScatter",
                        mybir.AluOpType.add,
                        replica_groups=[list(range(n_cores))],
                        ins=[cc_in[:]],
                        outs=[cc_out[:]],
                    )
                sdma.dma_start(
                    v_sb[0:1, :],
                    cc_out[:, :].rearrange("(one a) b -> one (a b)", one=1),
                )
            else:
                # phase 1 (replicated W): v = dec @ W on this core
                dec_sb = const.tile([bl, h], F32, tag="dec_sb")
                decT = const.tile([128, och * bl], F32, tag="decT")
                sdma.dma_start(dec_sb[:], dec[:, :])

                for oc in range(och):
                    dT_ps = psS.tile([128, bl], F32, tag="small")
                    nc.tensor.transpose(
                        dT_ps[:], dec_sb[:, oc * 128:(oc + 1) * 128],
                        ident[0:bl, 0:bl]
                    )
                    nc.vector.tensor_copy(
                        decT[:, oc * bl:(oc + 1) * bl], dT_ps[:]
                    )

                w_tiles = []
                for oc in range(och):
                    wt = wpool.tile([128, h], F32, tag=f"w{oc}")
                    sdma.dma_start(wt[:], w[oc * 128:(oc + 1) * 128, :])
                    w_tiles.append(wt)

                for b in range(bl):
                    for hh in range(nhh):
                        v_ps = psA.tile([1, 512], F32, tag="work")
                        for oc in range(och):
                            nc.tensor.matmul(
                                v_ps[:],
                                decT[:, oc * bl + b: oc * bl + b + 1],
                                w_tiles[oc][:, hh * 512:(hh + 1) * 512],
                                start=(oc == 0),
                                stop=(oc == och - 1),
                            )
                        nc.vector.tensor_copy(
                            v_sb[:, b * h + hh * 512: b * h + (hh + 1) * 512],
                            v_ps[:]
                        )

            # phase 2: broadcast v[b] across all 128 partitions
            for b in range(bl):
                for hh in range(nhh):
                    vb_ps = psA.tile([128, 512], F32, tag="work")
                    nc.tensor.matmul(
                        vb_ps[:],
                        ones[0:1, 0:128],
                        v_sb[0:1, b * h + hh * 512: b * h + (hh + 1) * 512],
                        start=True,
                        stop=True,
                    )
                    for rr in range(vrep):
                        nc.scalar.copy(
                            vb_cur[:, (b * vrep + rr) * h + hh * 512:
                                   (b * vrep + rr) * h + (hh + 1) * 512], vb_ps[:]
                        )
            return vb_cur

        # ---- phase 3+4: stream encoder, fused dot, softmax ----
        pending = []
        if not startup_in_loop:
            vb_rep = do_startup(0)
        for _rep in range(repeat):
            if startup_in_loop:
                vb_rep = do_startup(_rep)
            if staged:
                pending = _phase34_staged(
                    nc, tc, bl, t, h, tch, enc, attn, encp, scr, sm, outp,
                    psS, epool, vb_rep, ones, ident, pending, pair2=pair2)
                continue
            if dmaonly:
                # timing diagnostic: pure DMA stream, no consumer
                for b in range(bl):
                    for tcix in range(tch):
                        et = encp.tile([128, h], F32, tag="enc")
                        nc.sync.dma_start(
                            et[:], enc[b, tcix * 128:(tcix + 1) * 128, :])
            elif pair:
                _phase34_pair(nc, tc, bl, t, h, tch, enc, attn, encp, scr, sm,
                              outp, psS, epool, vb_rep, ones, ident)
            else:
                _phase34(nc, tc, bl, t, h, tch, enc, attn, encp, scr, sm, outp,
                         psS, epool, vb_rep, ones, ident, fuse=fuse,
                         pool_every=pool_every, act_copies=act_copies,
                         stt2=stt2, sm2=sm2)
        for f in pending:
            f()

    nc.compile()
    return nc


def _phase34(nc, tc, bl, t, h, tch, enc, attn, encp, scr, sm, outp, psS,
             epool, vb_all, ones, ident, fuse=False, pool_every=0,
             act_copies=False, stt2=False, sm2=False):
    if True:
        for b in range(bl):
            vb = vb_all[:, b * h:(b + 1) * h]
            e_t = epool.tile([128, tch], F32, tag=f"e{b}")
            for tcix in range(tch):
                et = encp.tile([128, h], F32, tag="enc")
                nc.sync.dma_start(et[:], enc[b, tcix * 128:(tcix + 1) * 128, :])
                sc = scr.tile([128, h], F32, tag="scr")
                use_pool = pool_every and (tcix % pool_every == pool_every - 1)
                if fuse and use_pool:
                    # offload this chunk's mul to gpsimd, accumulate on ACT
                    nc.gpsimd.tensor_mul(sc[:], et[:], vb)
                    dump = scr.tile([128, h], F32, tag="dump")
                    nc.scalar.activation(
                        dump[:], sc[:], mybir.ActivationFunctionType.Copy,
                        bias=0.0, scale=1.0,
                        accum_out=e_t[:, tcix: tcix + 1],
                    )
                elif fuse:
                    # single DVE op: sc = (et*1.0)*vb, e_col = sum_h(sc)
                    # (tensor_tensor_reduce and the gpsimd equivalent fail in
                    # neuronx-cc / at runtime; InstTensorScalarPtr works)
                    nc.vector.scalar_tensor_tensor(
                        out=sc[:], in0=et[:], scalar=1.0, in1=vb,
                        op0=mybir.AluOpType.mult, op1=mybir.AluOpType.mult,
                        accum_out=e_t[:, tcix: tcix + 1],
                    )
                    if stt2:
                        sc2 = scr.tile([128, h], F32, tag="scr2")
                        ed = epool.tile([128, tch], F32, tag=f"ed{b}")
                        nc.vector.scalar_tensor_tensor(
                            out=sc2[:], in0=et[:], scalar=1.0, in1=vb,
                            op0=mybir.AluOpType.mult, op1=mybir.AluOpType.mult,
                            accum_out=ed[:, tcix: tcix + 1],
                        )
                else:
                    nc.vector.tensor_mul(sc[:], et[:], vb)
                    dump = scr.tile([128, h], F32, tag="dump")
                    nc.scalar.activation(
                        dump[:], sc[:], mybir.ActivationFunctionType.Copy,
                        bias=0.0, scale=1.0,
                        accum_out=e_t[:, tcix: tcix + 1],
                    )

            _softmax_batch(nc, b, tch, attn, sm, outp, psS, e_t, ones, ident,
                           act_copies=act_copies)
            if sm2:
                _softmax_batch(nc, b, tch, attn, sm, outp, psS, e_t, ones,
                               ident, act_copies=act_copies)


def _softmax_stages(nc, b, tch, attn, sm, outp, psS, e_t, ones, ident,
                    out_dma="scalar"):
    """Softmax for one batch, split into 5 emission units so the caller can
    interleave them into the next batch's stt stream: each unit's cross-
    engine inputs are produced several chunks before the unit is emitted,
    so the in-order DVE queue never stalls on PE/ACT round trips."""
    st = {}

    def s1():
        m1 = sm.tile([128, 1], F32, tag="m1")
        nc.vector.tensor_reduce(out=m1[:], in_=e_t[:, :],
                                axis=mybir.AxisListType.X,
                                op=mybir.AluOpType.max)
        m1T = psS.tile([1, 128], F32, tag="small")
        nc.tensor.transpose(m1T[:], m1[:], ident[:, :])
        st["m1T"] = m1T

    def s2():
        M = sm.tile([1, 1], F32, tag="M")
        nc.vector.tensor_reduce(out=M[:], in_=st["m1T"][0:1, :],
                                axis=mybir.AxisListType.X,
                                op=mybir.AluOpType.max)
        Mb_ps = psS.tile([128, 1], F32, tag="small")
        nc.tensor.matmul(Mb_ps[:], ones[0:1, 0:128], M[0:1, 0:1],
                         start=True, stop=True)
        negM = sm.tile([128, 1], F32, tag="negM")
        nc.scalar.mul(negM[:], Mb_ps[:], -1.0)
        st["negM"] = negM

    def s3():
        p_b = sm.tile([128, tch], F32, tag="p")
        s1v = sm.tile([128, 1], F32, tag="s1")
        nc.scalar.activation(
            p_b[:], e_t[:, :], mybir.ActivationFunctionType.Exp,
            bias=st["negM"][:, 0:1], scale=1.0, accum_out=s1v[:],
        )
        s1T = psS.tile([1, 128], F32, tag="small")
        nc.tensor.transpose(s1T[:], s1v[:], ident[:, :])
        st["p"] = p_b
        st["s1T"] = s1T

    def s4():
        S = sm.tile([1, 1], F32, tag="S")
        nc.vector.tensor_reduce(out=S[:], in_=st["s1T"][0:1, :],
                                axis=mybir.AxisListType.X,
                                op=mybir.AluOpType.add)
        R = sm.tile([1, 1], F32, tag="R")
        nc.vector.reciprocal(R[:], S[:])
        Rb_ps = psS.tile([128, 1], F32, tag="small")
        nc.tensor.matmul(Rb_ps[:], ones[0:1, 0:128], R[0:1, 0:1],
                         start=True, stop=True)
        Rb = sm.tile([128, 1], F32, tag="Rbs")
        nc.scalar.copy(Rb[:], Rb_ps[:])
        st["Rb"] = Rb

    def s5():
        a_b = sm.tile([128, tch], F32, tag="a")
        nc.vector.tensor_scalar_mul(a_b[:], st["p"][:], st["Rb"][:, 0:1])
        aT_ps = psS.tile([tch, 128], F32, tag="small")
        nc.tensor.transpose(aT_ps[:], a_b[:], ident[:, :])
        aT = outp.tile([tch, 128], F32, tag="aTs")
        nc.scalar.copy(aT[:], aT_ps[:])
        getattr(nc, out_dma).dma_start(
            attn[b].rearrange("(c p) -> c p", p=128), aT[:]
        )

    return [s1, s2, s3, s4, s5]


def _phase34_staged(nc, tc, bl, t, h, tch, enc, attn, encp, scr, sm, outp,
                    psS, epool, vb_all, ones, ident, pending, pair2=False):
    """Fused stream with the previous batches' softmax stages interleaved.
    `pending` carries not-yet-emitted stages across batches AND reps.
    pair2: one 1MB DMA per 256 encoder rows (partition p holds rows
    tp*256+p and tp*256+128+p), two stt ops per tile — halves the DMA count
    and the DVE semaphore waits; e_t column layout is unchanged."""
    for b in range(bl):
        vb = vb_all[:, b * h:(b + 1) * h]
        e_t = epool.tile([128, tch], F32, tag=f"e{b}")
        if pair2:
            ipoints = (1, 2, 3, 5, 6)
            for tp in range(tch // 2):
                et = encp.tile([128, 2 * h], F32, tag="enc")
                nc.sync.dma_start(
                    et[:].rearrange("p (n h) -> p n h", n=2),
                    enc[b, tp * 256:(tp + 1) * 256, :].rearrange(
                        "(n p) h -> p n h", p=128),
                )
                for n in range(2):
                    sc = scr.tile([128, h], F32, tag="scr")
                    nc.vector.scalar_tensor_tensor(
                        out=sc[:], in0=et[:, n * h:(n + 1) * h], scalar=1.0,
                        in1=vb,
                        op0=mybir.AluOpType.mult, op1=mybir.AluOpType.mult,
                        accum_out=e_t[:, 2 * tp + n: 2 * tp + n + 1],
                    )
                if tp in ipoints and pending:
                    pending.pop(0)()
        else:
            ipoints = (2, 5, 8, 11, 14)
            for tcix in range(tch):
                et = encp.tile([128, h], F32, tag="enc")
                nc.sync.dma_start(et[:], enc[b, tcix * 128:(tcix + 1) * 128, :])
                sc = scr.tile([128, h], F32, tag="scr")
                nc.vector.scalar_tensor_tensor(
                    out=sc[:], in0=et[:], scalar=1.0, in1=vb,
                    op0=mybir.AluOpType.mult, op1=mybir.AluOpType.mult,
                    accum_out=e_t[:, tcix: tcix + 1],
                )
                if tcix in ipoints and pending:
                    pending.pop(0)()
        pending.extend(
            _softmax_stages(nc, b, tch, attn, sm, outp, psS, e_t, ones, ident)
        )
    return pending


def _phase34_pair(nc, tc, bl, t, h, tch, enc, attn, encp, scr, sm, outp, psS,
                  epool, vb_all, ones, ident):
    for b in range(bl):
        vb2 = vb_all[:, b * 2 * h:(b + 1) * 2 * h]
        e_t = epool.tile([128, tch], F32, tag=f"e{b}")
        for tp in range(tch // 2):
            et = encp.tile([128, 2 * h], F32, tag="enc")
            nc.sync.dma_start(
                et[:].rearrange("p (n h) -> p n h", n=2),
                enc[b, tp * 256:(tp + 1) * 256, :].rearrange(
                    "(n p) h -> p n h", p=128),
            )
            sc = scr.tile([128, 2 * h], F32, tag="scr")
            nc.vector.tensor_mul(sc[:], et[:], vb2)
            for n in range(2):
                dump = scr.tile([128, h], F32, tag="dump")
                nc.scalar.activation(
                    dump[:], sc[:, n * h:(n + 1) * h],
                    mybir.ActivationFunctionType.Copy,
                    bias=0.0, scale=1.0,
                    accum_out=e_t[:, 2 * tp + n: 2 * tp + n + 1],
                )
        _softmax_batch(nc, b, tch, attn, sm, outp, psS, e_t, ones, ident)


def _softmax_batch(nc, b, tch, attn, sm, outp, psS, e_t, ones, ident,
                   act_copies=False):
    if True:
        if True:
            # softmax over the [128, tch] energies of this batch
            e_b = e_t[:, :]

            m1 = sm.tile([128, 1], F32, tag="m1")
            nc.vector.tensor_reduce(
                out=m1[:], in_=e_b, axis=mybir.AxisListType.X, op=mybir.AluOpType.max
            )
            m1T = psS.tile([1, 128], F32, tag="small")
            nc.tensor.transpose(m1T[:], m1[:], ident[:, :])
            M = sm.tile([1, 1], F32, tag="M")
            nc.vector.tensor_reduce(
                out=M[:], in_=m1T[0:1, :], axis=mybir.AxisListType.X,
                op=mybir.AluOpType.max,
            )
            Mb_ps = psS.tile([128, 1], F32, tag="small")
            nc.tensor.matmul(Mb_ps[:], ones[0:1, 0:128], M[0:1, 0:1],
                             start=True, stop=True)
            negM = sm.tile([128, 1], F32, tag="negM")
            nc.scalar.mul(negM[:], Mb_ps[:], -1.0)

            p_b = sm.tile([128, tch], F32, tag="p")
            s1 = sm.tile([128, 1], F32, tag="s1")
            nc.scalar.activation(
                p_b[:], e_b, mybir.ActivationFunctionType.Exp,
                bias=negM[:, 0:1], scale=1.0, accum_out=s1[:],
            )
            s1T = psS.tile([1, 128], F32, tag="small")
            nc.tensor.transpose(s1T[:], s1[:], ident[:, :])
            S = sm.tile([1, 1], F32, tag="S")
            nc.vector.tensor_reduce(
                out=S[:], in_=s1T[0:1, :], axis=mybir.AxisListType.X,
                op=mybir.AluOpType.add,
            )
            R = sm.tile([1, 1], F32, tag="R")
            nc.vector.reciprocal(R[:], S[:])
            Rb_ps = psS.tile([128, 1], F32, tag="small")
            nc.tensor.matmul(Rb_ps[:], ones[0:1, 0:128], R[0:1, 0:1],
                             start=True, stop=True)
            Rb = sm.tile([128, 1], F32, tag="Rbs")
            nc.scalar.copy(Rb[:], Rb_ps[:])

            a_b = sm.tile([128, tch], F32, tag="a")
            nc.vector.tensor_scalar_mul(a_b[:], p_b[:], Rb[:, 0:1])

            aT_ps = psS.tile([tch, 128], F32, tag="small")
            nc.tensor.transpose(aT_ps[:], a_b[:], ident[:, :])
            aT = outp.tile([tch, 128], F32, tag="aTs")
            (nc.scalar.copy if act_copies else nc.vector.tensor_copy)(
                aT[:], aT_ps[:])
            nc.sync.dma_start(
                attn[b].rearrange("(c p) -> c p", p=128), aT[:]
            )


_NC_CACHE = {}


WSHARD = True  # shard W 8-ways + ReduceScatter partial v (saves 3.5MB/core DMA)
# production build config (see module docstring for what each flag buys)
BEST = dict(wshard=WSHARD, fuse=True, dbuf=True, act_copies=True,
            staged=True, enc_bufs=30, scr_bufs=2)


def _get_nc():
    if "nc" not in _NC_CACHE:
        _NC_CACHE["nc"] = build_kernel(**BEST)
    return _NC_CACHE["nc"]


def shard_inputs(decoder_output, encoder_outputs, W, wshard=False):
    """Per-core input dicts for the chosen W distribution scheme."""
    maps = []
    for c in range(NCORES):
        sl = slice(c * BL, (c + 1) * BL)
        m = {"enc": np.ascontiguousarray(encoder_outputs[sl], dtype=np.float32)}
        if wshard:
            m["dec"] = np.ascontiguousarray(
                decoder_output[:, c * 128:(c + 1) * 128], dtype=np.float32)
            m["w"] = np.ascontiguousarray(
                W[c * 128:(c + 1) * 128, :], dtype=np.float32)
        else:
            m["dec"] = np.ascontiguousarray(decoder_output[sl], dtype=np.float32)
            m["w"] = np.ascontiguousarray(W, dtype=np.float32)
        maps.append(m)
    return maps


def nc_is_wshard(nc):
    for alloc in nc.m.functions[0].allocations:
        if isinstance(alloc, mybir.MemoryLocationSet) and \
                alloc.memorylocations[0].name == "w":
            return tuple(alloc.tensor_shape) == (128, H)
    return False


def run_sharded(decoder_output, encoder_outputs, W, trace=False, nc=None, **kw):
    if nc is None:
        nc = _get_nc()
    in_maps = shard_inputs(decoder_output, encoder_outputs, W,
                           wshard=nc_is_wshard(nc))
    res = run_bass_kernel_spmd(nc, in_maps, list(range(NCORES)), trace=trace, **kw)
    attn = np.concatenate([res.results[c]["attn"] for c in range(NCORES)], axis=0)
    return attn, res


def kernel(decoder_output, encoder_outputs, W, b=None, **_unused):
    # b (the Linear bias) shifts every energy of a batch equally -> cancels in
    # softmax; it is deliberately unused.
    attn, _ = run_sharded(decoder_output, encoder_outputs, W)
    return attn.reshape(B, T, 1).astype(np.float32)



# revision 19
# speedup vs baseline: 1.1072x; 1.1072x over previous
"""Trainium2 Bass kernel for DecoderAttn ('general' attention score + softmax).

Reference computation (per batch b):
    energies[t] = dec[b] . (enc[b,t] @ W.T + bias)
    attn = softmax(energies over t)

Algebraic rewrite used here:
    energies[t] = enc[b,t] . (W.T @ dec[b])  +  (bias . dec[b])
The second term is constant over t, so it drops out of the softmax exactly.
This turns an O(B*T*H^2) matmul chain into an O(B*H^2 + B*T*H) streaming
problem: precompute v[b] = (dec @ W)[b] on the tensor engine, stream the
encoder through one fused DVE op per [128,1024] tile, then a tiny softmax.

Sharding: data-parallel over batch B=32 across 8 NeuronCores (4 batches per
core). W is sharded 8-ways by rows (o-chunks): each core computes a partial
v for all 32 batches over its 128-row W slice, and a ReduceScatter-add hands
core c exactly its own 4 batches' v — cutting per-core DMA from 36.1MB to
~32.7MB vs replicating W.

Performance structure (measured via repeat-slope benching + TimelineSim):
  - fuse: the per-tile dot is ONE DVE scalar_tensor_tensor (out=et*vb,
    accum_out=column of e_t) at ~1.0us/tile — replaces DVE mul + ACT
    accumulate, freeing the ACT engine (~-18us).
    (tensor_tensor_reduce and the gpsimd equivalent crash/fail to compile.)
  - dbuf: the v chain (dec/W load, matmuls, ReduceScatter, broadcast)
    double-buffers its SBUF state across reps, so rep r+1's collective
    overlaps rep r's stream; the RS otherwise costs ~19us/rep (~-14us).
  - staged: the per-batch softmax is split into 5 stages interleaved into
    the NEXT batch's stt stream. The in-order DVE queue otherwise stalls
    ~4us/batch on softmax cross-engine round trips (~-12us).
  - act_copies: startup/softmax PSUM->SBUF copies go to the ACT engine,
    keeping the DVE queue pure stt.
With all of these the steady state is DMA-paced: per-core traffic is
33.55MB enc + 0.5MB W + ~0.2MB dec/cc per workload at the ~358GB/s
per-core HBM ceiling (~95us theoretical floor; DMA-only variant measures
within a few us of the full kernel).
"""

import numpy as np
from contextlib import ExitStack

import concourse.bass as bass
import concourse.tile as tile
from concourse import bacc, mybir, masks
from concourse.bass_utils import run_bass_kernel_spmd

F32 = mybir.dt.float32

B, T, H = 32, 2048, 1024
NCORES = 8
BL = B // NCORES           # batches per core
TCH = T // 128             # 128-row t-chunks per batch
OCH = H // 128             # 128-row o-chunks of W


def build_kernel(bl=BL, t=T, h=H, enc_bufs=24, repeat=1, scr_bufs=4, pair=False,
                 wshard=False, n_cores=NCORES, startup_in_loop=False,
                 startup_dma="scalar", fuse=False, pool_every=0,
                 no_rs=False, dmaonly=False, dbuf=False, act_copies=False,
                 stt2=False, sm2=False, staged=False, pair2=False,
                 sm_bufs=2, eb=2, ps_bufs=3):
    tch = t // 128
    och = h // 128
    nhh = h // 512  # 512-wide halves of the H free dim for matmul N-limit

    nc = bacc.Bacc("TRN2", target_bir_lowering=False, debug=False)

    if wshard:
        # every core gets: dec columns for ITS o-chunk [B, 128], W rows for
        # ITS o-chunk [128, h]; partial v is summed across cores with a
        # ReduceScatter that hands core c exactly its 4 batches' v.
        nb = bl * n_cores
        dec = nc.dram_tensor("dec", [nb, 128], F32, kind="ExternalInput")
        w = nc.dram_tensor("w", [128, h], F32, kind="ExternalInput")
    else:
        dec = nc.dram_tensor("dec", [bl, h], F32, kind="ExternalInput")
        w = nc.dram_tensor("w", [h, h], F32, kind="ExternalInput")
    enc = nc.dram_tensor("enc", [bl, t, h], F32, kind="ExternalInput")
    attn = nc.dram_tensor("attn", [bl, t], F32, kind="ExternalOutput")

    with tile.TileContext(nc) as tc, ExitStack() as ctx:
        const = ctx.enter_context(tc.tile_pool(name="const", bufs=1))
        wpool = ctx.enter_context(tc.tile_pool(name="wpool", bufs=1))
        encp = ctx.enter_context(tc.tile_pool(name="encp", bufs=enc_bufs))
        scr = ctx.enter_context(tc.tile_pool(name="scr", bufs=scr_bufs))
        sm = ctx.enter_context(tc.tile_pool(name="sm", bufs=sm_bufs))
        outp = ctx.enter_context(tc.tile_pool(name="outp", bufs=2))
        psA = ctx.enter_context(tc.tile_pool(name="psA", bufs=2, space="PSUM"))
        psS = ctx.enter_context(tc.tile_pool(name="psS", bufs=ps_bufs, space="PSUM"))

        sdma = getattr(nc, startup_dma)

        # ---- constants ----
        ident = const.tile([128, 128], F32)
        masks.make_identity(nc, ident[:])
        ones = const.tile([1, 128], F32)
        nc.gpsimd.memset(ones[:], 1.0)

        # long-lived state
        epool = ctx.enter_context(tc.tile_pool(name="epool", bufs=eb))
        vrep = 2 if pair else 1
        if not dbuf:
            vb_all = const.tile([128, bl * vrep * h], F32)  # v[b] bcast

        copy_small = nc.scalar.copy if act_copies else nc.vector.tensor_copy

        def do_startup(rep):
            # dbuf alternates the vb buffer across reps so rep r+1's whole
            # v chain (incl. the ReduceScatter) overlaps rep r's enc stream
            if dbuf:
                vb_cur = const.tile([128, bl * vrep * h], F32,
                                    tag=f"vball{rep % 2}")
            else:
                vb_cur = vb_all
            v_sb = const.tile([1, bl * h], F32, tag="v_sb")  # rows on partition 0
            if wshard:
                # phase 1 (sharded W): partial v over this core's o-chunk,
                # ReduceScatter-add across cores
                dec_sb = const.tile([nb, 128], F32, tag="dec_sb")
                decT = const.tile([128, nb], F32, tag="decT")
                pv_sb = const.tile([nb, h], F32, tag="pv_sb")
                cc_in = nc.dram_tensor(f"cc_in{rep}", [nb, h], F32)
                cc_out = nc.dram_tensor(f"cc_out{rep}", [bl, h], F32)

                sdma.dma_start(dec_sb[:], dec[:, :])
                dT_ps = psS.tile([128, nb], F32, tag="small")
                nc.tensor.transpose(dT_ps[:], dec_sb[:, :], ident[0:nb, 0:nb])
                copy_small(decT[:, :], dT_ps[:])
                wt = wpool.tile([128, h], F32, tag="w0")
                sdma.dma_start(wt[:], w[:, :])
                for hh in range(nhh):
                    pv_ps = psA.tile([nb, 512], F32, tag="work")
                    nc.tensor.matmul(
                        pv_ps[:], decT[:, :], wt[:, hh * 512:(hh + 1) * 512],
                        start=True, stop=True,
                    )
                    copy_small(
                        pv_sb[:, hh * 512:(hh + 1) * 512], pv_ps[:]
                    )
                sdma.dma_start(cc_in[:, :], pv_sb[:])
                if no_rs:
                    # timing diagnostic: skip the collective (WRONG results)
                    sdma.dma_start(cc_out[:, :], cc_in[0:bl, :])
                else:
                    nc.gpsimd.collective_compute(
                        "ReduceScatter",
                        mybir.AluOpType.add,
                        replica_groups=[list(range(n_cores))],
                        ins=[cc_in[:]],
                        outs=[cc_out[:]],
                    )
                sdma.dma_start(
                    v_sb[0:1, :],
                    cc_out[:, :].rearrange("(one a) b -> one (a b)", one=1),
                )
            else:
                # phase 1 (replicated W): v = dec @ W on this core
                dec_sb = const.tile([bl, h], F32, tag="dec_sb")
                decT = const.tile([128, och * bl], F32, tag="decT")
                sdma.dma_start(dec_sb[:], dec[:, :])

                for oc in range(och):
                    dT_ps = psS.tile([128, bl], F32, tag="small")
                    nc.tensor.transpose(
                        dT_ps[:], dec_sb[:, oc * 128:(oc + 1) * 128],
                        ident[0:bl, 0:bl]
                    )
                    nc.vector.tensor_copy(
                        decT[:, oc * bl:(oc + 1) * bl], dT_ps[:]
                    )

                w_tiles = []
                for oc in range(och):
                    wt = wpool.tile([128, h], F32, tag=f"w{oc}")
                    sdma.dma_start(wt[:], w[oc * 128:(oc + 1) * 128, :])
                    w_tiles.append(wt)

                for b in range(bl):
                    for hh in range(nhh):
                        v_ps = psA.tile([1, 512], F32, tag="work")
                        for oc in range(och):
                            nc.tensor.matmul(
                                v_ps[:],
                                decT[:, oc * bl + b: oc * bl + b + 1],
                                w_tiles[oc][:, hh * 512:(hh + 1) * 512],
                                start=(oc == 0),
                                stop=(oc == och - 1),
                            )
                        nc.vector.tensor_copy(
                            v_sb[:, b * h + hh * 512: b * h + (hh + 1) * 512],
                            v_ps[:]
                        )

            # phase 2: broadcast v[b] across all 128 partitions
            for b in range(bl):
                for hh in range(nhh):
                    vb_ps = psA.tile([128, 512], F32, tag="work")
                    nc.tensor.matmul(
                        vb_ps[:],
                        ones[0:1, 0:128],
                        v_sb[0:1, b * h + hh * 512: b * h + (hh + 1) * 512],
                        start=True,
                        stop=True,
                    )
                    for rr in range(vrep):
                        nc.scalar.copy(
                            vb_cur[:, (b * vrep + rr) * h + hh * 512:
                                   (b * vrep + rr) * h + (hh + 1) * 512], vb_ps[:]
                        )
            return vb_cur

        # ---- phase 3+4: stream encoder, fused dot, softmax ----
        pending = []
        if not startup_in_loop:
            vb_rep = do_startup(0)
        for _rep in range(repeat):
            if startup_in_loop:
                vb_rep = do_startup(_rep)
            if staged:
                pending = _phase34_staged(
                    nc, tc, bl, t, h, tch, enc, attn, encp, scr, sm, outp,
                    psS, epool, vb_rep, ones, ident, pending, pair2=pair2)
                continue
            if dmaonly:
                # timing diagnostic: pure DMA stream, no consumer
                for b in range(bl):
                    for tcix in range(tch):
                        et = encp.tile([128, h], F32, tag="enc")
                        nc.sync.dma_start(
                            et[:], enc[b, tcix * 128:(tcix + 1) * 128, :])
            elif pair:
                _phase34_pair(nc, tc, bl, t, h, tch, enc, attn, encp, scr, sm,
                              outp, psS, epool, vb_rep, ones, ident)
            else:
                _phase34(nc, tc, bl, t, h, tch, enc, attn, encp, scr, sm, outp,
                         psS, epool, vb_rep, ones, ident, fuse=fuse,
                         pool_every=pool_every, act_copies=act_copies,
                         stt2=stt2, sm2=sm2)
        for f in pending:
            f()

    nc.compile()
    return nc


def _phase34(nc, tc, bl, t, h, tch, enc, attn, encp, scr, sm, outp, psS,
             epool, vb_all, ones, ident, fuse=False, pool_every=0,
             act_copies=False, stt2=False, sm2=False):
    if True:
        for b in range(bl):
            vb = vb_all[:, b * h:(b + 1) * h]
            e_t = epool.tile([128, tch], F32, tag=f"e{b}")
            for tcix in range(tch):
                et = encp.tile([128, h], F32, tag="enc")
                nc.sync.dma_start(et[:], enc[b, tcix * 128:(tcix + 1) * 128, :])
                sc = scr.tile([128, h], F32, tag="scr")
                use_pool = pool_every and (tcix % pool_every == pool_every - 1)
                if fuse and use_pool:
                    # offload this chunk's mul to gpsimd, accumulate on ACT
                    nc.gpsimd.tensor_mul(sc[:], et[:], vb)
                    dump = scr.tile([128, h], F32, tag="dump")
                    nc.scalar.activation(
                        dump[:], sc[:], mybir.ActivationFunctionType.Copy,
                        bias=0.0, scale=1.0,
                        accum_out=e_t[:, tcix: tcix + 1],
                    )
                elif fuse:
                    # single DVE op: sc = (et*1.0)*vb, e_col = sum_h(sc)
                    # (tensor_tensor_reduce and the gpsimd equivalent fail in
                    # neuronx-cc / at runtime; InstTensorScalarPtr works)
                    nc.vector.scalar_tensor_tensor(
                        out=sc[:], in0=et[:], scalar=1.0, in1=vb,
                        op0=mybir.AluOpType.mult, op1=mybir.AluOpType.mult,
                        accum_out=e_t[:, tcix: tcix + 1],
                    )
                    if stt2:
                        sc2 = scr.tile([128, h], F32, tag="scr2")
                        ed = epool.tile([128, tch], F32, tag=f"ed{b}")
                        nc.vector.scalar_tensor_tensor(
                            out=sc2[:], in0=et[:], scalar=1.0, in1=vb,
                            op0=mybir.AluOpType.mult, op1=mybir.AluOpType.mult,
                            accum_out=ed[:, tcix: tcix + 1],
                        )
                else:
                    nc.vector.tensor_mul(sc[:], et[:], vb)
                    dump = scr.tile([128, h], F32, tag="dump")
                    nc.scalar.activation(
                        dump[:], sc[:], mybir.ActivationFunctionType.Copy,
                        bias=0.0, scale=1.0,
                        accum_out=e_t[:, tcix: tcix + 1],
                    )

            _softmax_batch(nc, b, tch, attn, sm, outp, psS, e_t, ones, ident,
                           act_copies=act_copies)
            if sm2:
                _softmax_batch(nc, b, tch, attn, sm, outp, psS, e_t, ones,
                               ident, act_copies=act_copies)


def _softmax_stages(nc, b, tch, attn, sm, outp, psS, e_t, ones, ident,
                    out_dma="scalar"):
    """Softmax for one batch, split into 5 emission units so the caller can
    interleave them into the next batch's stt stream: each unit's cross-
    engine inputs are produced several chunks before the unit is emitted,
    so the in-order DVE queue never stalls on PE/ACT round trips."""
    st = {}

    def s1():
        m1 = sm.tile([128, 1], F32, tag="m1")
        nc.vector.tensor_reduce(out=m1[:], in_=e_t[:, :],
                                axis=mybir.AxisListType.X,
                                op=mybir.AluOpType.max)
        m1T = psS.tile([1, 128], F32, tag="small")
        nc.tensor.transpose(m1T[:], m1[:], ident[:, :])
        st["m1T"] = m1T

    def s2():
        M = sm.tile([1, 1], F32, tag="M")
        nc.vector.tensor_reduce(out=M[:], in_=st["m1T"][0:1, :],
                                axis=mybir.AxisListType.X,
                                op=mybir.AluOpType.max)
        Mb_ps = psS.tile([128, 1], F32, tag="small")
        nc.tensor.matmul(Mb_ps[:], ones[0:1, 0:128], M[0:1, 0:1],
                         start=True, stop=True)
        negM = sm.tile([128, 1], F32, tag="negM")
        nc.scalar.mul(negM[:], Mb_ps[:], -1.0)
        st["negM"] = negM

    def s3():
        p_b = sm.tile([128, tch], F32, tag="p")
        s1v = sm.tile([128, 1], F32, tag="s1")
        nc.scalar.activation(
            p_b[:], e_t[:, :], mybir.ActivationFunctionType.Exp,
            bias=st["negM"][:, 0:1], scale=1.0, accum_out=s1v[:],
        )
        s1T = psS.tile([1, 128], F32, tag="small")
        nc.tensor.transpose(s1T[:], s1v[:], ident[:, :])
        st["p"] = p_b
        st["s1T"] = s1T

    def s4():
        S = sm.tile([1, 1], F32, tag="S")
        nc.vector.tensor_reduce(out=S[:], in_=st["s1T"][0:1, :],
                                axis=mybir.AxisListType.X,
                                op=mybir.AluOpType.add)
        R = sm.tile([1, 1], F32, tag="R")
        nc.vector.reciprocal(R[:], S[:])
        Rb_ps = psS.tile([128, 1], F32, tag="small")
        nc.tensor.matmul(Rb_ps[:], ones[0:1, 0:128], R[0:1, 0:1],
                         start=True, stop=True)
        Rb = sm.tile([128, 1], F32, tag="Rbs")
        nc.scalar.copy(Rb[:], Rb_ps[:])
        st["Rb"] = Rb

    def s5():
        a_b = sm.tile([128, tch], F32, tag="a")
        nc.vector.tensor_scalar_mul(a_b[:], st["p"][:], st["Rb"][:, 0:1])
        aT_ps = psS.tile([tch, 128], F32, tag="small")
        nc.tensor.transpose(aT_ps[:], a_b[:], ident[:, :])
        aT = outp.tile([tch, 128], F32, tag="aTs")
        nc.scalar.copy(aT[:], aT_ps[:])
        getattr(nc, out_dma).dma_start(
            attn[b].rearrange("(c p) -> c p", p=128), aT[:]
        )

    return [s1, s2, s3, s4, s5]


def _phase34_staged(nc, tc, bl, t, h, tch, enc, attn, encp, scr, sm, outp,
                    psS, epool, vb_all, ones, ident, pending, pair2=False):
    """Fused stream with the previous batches' softmax stages interleaved.
    `pending` carries not-yet-emitted stages across batches AND reps.
    pair2: one 1MB DMA per 256 encoder rows (partition p holds rows
    tp*256+p and tp*256+128+p), two stt ops per tile — halves the DMA count
    and the DVE semaphore waits; e_t column layout is unchanged."""
    for b in range(bl):
        vb = vb_all[:, b * h:(b + 1) * h]
        e_t = epool.tile([128, tch], F32, tag=f"e{b}")
        if pair2:
            ipoints = (1, 2, 3, 5, 6)
            for tp in range(tch // 2):
                et = encp.tile([128, 2 * h], F32, tag="enc")
                nc.sync.dma_start(
                    et[:].rearrange("p (n h) -> p n h", n=2),
                    enc[b, tp * 256:(tp + 1) * 256, :].rearrange(
                        "(n p) h -> p n h", p=128),
                )
                for n in range(2):
                    sc = scr.tile([128, h], F32, tag="scr")
                    nc.vector.scalar_tensor_tensor(
                        out=sc[:], in0=et[:, n * h:(n + 1) * h], scalar=1.0,
                        in1=vb,
                        op0=mybir.AluOpType.mult, op1=mybir.AluOpType.mult,
                        accum_out=e_t[:, 2 * tp + n: 2 * tp + n + 1],
                    )
                if tp in ipoints and pending:
                    pending.pop(0)()
        else:
            ipoints = getattr(_phase34_staged, "IPOINTS", (2, 5, 8, 11, 14))
            for tcix in range(tch):
                et = encp.tile([128, h], F32, tag="enc")
                nc.sync.dma_start(et[:], enc[b, tcix * 128:(tcix + 1) * 128, :])
                sc = scr.tile([128, h], F32, tag="scr")
                nc.vector.scalar_tensor_tensor(
                    out=sc[:], in0=et[:], scalar=1.0, in1=vb,
                    op0=mybir.AluOpType.mult, op1=mybir.AluOpType.mult,
                    accum_out=e_t[:, tcix: tcix + 1],
                )
                if tcix in ipoints and pending:
                    pending.pop(0)()
        pending.extend(
            _softmax_stages(nc, b, tch, attn, sm, outp, psS, e_t, ones, ident)
        )
    return pending


def _phase34_pair(nc, tc, bl, t, h, tch, enc, attn, encp, scr, sm, outp, psS,
                  epool, vb_all, ones, ident):
    for b in range(bl):
        vb2 = vb_all[:, b * 2 * h:(b + 1) * 2 * h]
        e_t = epool.tile([128, tch], F32, tag=f"e{b}")
        for tp in range(tch // 2):
            et = encp.tile([128, 2 * h], F32, tag="enc")
            nc.sync.dma_start(
                et[:].rearrange("p (n h) -> p n h", n=2),
                enc[b, tp * 256:(tp + 1) * 256, :].rearrange(
                    "(n p) h -> p n h", p=128),
            )
            sc = scr.tile([128, 2 * h], F32, tag="scr")
            nc.vector.tensor_mul(sc[:], et[:], vb2)
            for n in range(2):
                dump = scr.tile([128, h], F32, tag="dump")
                nc.scalar.activation(
                    dump[:], sc[:, n * h:(n + 1) * h],
                    mybir.ActivationFunctionType.Copy,
                    bias=0.0, scale=1.0,
                    accum_out=e_t[:, 2 * tp + n: 2 * tp + n + 1],
                )
        _softmax_batch(nc, b, tch, attn, sm, outp, psS, e_t, ones, ident)


def _softmax_batch(nc, b, tch, attn, sm, outp, psS, e_t, ones, ident,
                   act_copies=False):
    if True:
        if True:
            # softmax over the [128, tch] energies of this batch
            e_b = e_t[:, :]

            m1 = sm.tile([128, 1], F32, tag="m1")
            nc.vector.tensor_reduce(
                out=m1[:], in_=e_b, axis=mybir.AxisListType.X, op=mybir.AluOpType.max
            )
            m1T = psS.tile([1, 128], F32, tag="small")
            nc.tensor.transpose(m1T[:], m1[:], ident[:, :])
            M = sm.tile([1, 1], F32, tag="M")
            nc.vector.tensor_reduce(
                out=M[:], in_=m1T[0:1, :], axis=mybir.AxisListType.X,
                op=mybir.AluOpType.max,
            )
            Mb_ps = psS.tile([128, 1], F32, tag="small")
            nc.tensor.matmul(Mb_ps[:], ones[0:1, 0:128], M[0:1, 0:1],
                             start=True, stop=True)
            negM = sm.tile([128, 1], F32, tag="negM")
            nc.scalar.mul(negM[:], Mb_ps[:], -1.0)

            p_b = sm.tile([128, tch], F32, tag="p")
            s1 = sm.tile([128, 1], F32, tag="s1")
            nc.scalar.activation(
                p_b[:], e_b, mybir.ActivationFunctionType.Exp,
                bias=negM[:, 0:1], scale=1.0, accum_out=s1[:],
            )
            s1T = psS.tile([1, 128], F32, tag="small")
            nc.tensor.transpose(s1T[:], s1[:], ident[:, :])
            S = sm.tile([1, 1], F32, tag="S")
            nc.vector.tensor_reduce(
                out=S[:], in_=s1T[0:1, :], axis=mybir.AxisListType.X,
                op=mybir.AluOpType.add,
            )
            R = sm.tile([1, 1], F32, tag="R")
            nc.vector.reciprocal(R[:], S[:])
            Rb_ps = psS.tile([128, 1], F32, tag="small")
            nc.tensor.matmul(Rb_ps[:], ones[0:1, 0:128], R[0:1, 0:1],
                             start=True, stop=True)
            Rb = sm.tile([128, 1], F32, tag="Rbs")
            nc.scalar.copy(Rb[:], Rb_ps[:])

            a_b = sm.tile([128, tch], F32, tag="a")
            nc.vector.tensor_scalar_mul(a_b[:], p_b[:], Rb[:, 0:1])

            aT_ps = psS.tile([tch, 128], F32, tag="small")
            nc.tensor.transpose(aT_ps[:], a_b[:], ident[:, :])
            aT = outp.tile([tch, 128], F32, tag="aTs")
            (nc.scalar.copy if act_copies else nc.vector.tensor_copy)(
                aT[:], aT_ps[:])
            nc.sync.dma_start(
                attn[b].rearrange("(c p) -> c p", p=128), aT[:]
            )


_NC_CACHE = {}


WSHARD = True  # shard W 8-ways + ReduceScatter partial v (saves 3.5MB/core DMA)
# production build config (see module docstring for what each flag buys)
BEST = dict(wshard=WSHARD, fuse=True, dbuf=True, act_copies=True,
            staged=True, enc_bufs=30, scr_bufs=2)


def _get_nc():
    if "nc" not in _NC_CACHE:
        _NC_CACHE["nc"] = build_kernel(**BEST)
    return _NC_CACHE["nc"]


def shard_inputs(decoder_output, encoder_outputs, W, wshard=False):
    """Per-core input dicts for the chosen W distribution scheme."""
    maps = []
    for c in range(NCORES):
        sl = slice(c * BL, (c + 1) * BL)
        m = {"enc": np.ascontiguousarray(encoder_outputs[sl], dtype=np.float32)}
        if wshard:
            m["dec"] = np.ascontiguousarray(
                decoder_output[:, c * 128:(c + 1) * 128], dtype=np.float32)
            m["w"] = np.ascontiguousarray(
                W[c * 128:(c + 1) * 128, :], dtype=np.float32)
        else:
            m["dec"] = np.ascontiguousarray(decoder_output[sl], dtype=np.float32)
            m["w"] = np.ascontiguousarray(W, dtype=np.float32)
        maps.append(m)
    return maps


def nc_is_wshard(nc):
    for alloc in nc.m.functions[0].allocations:
        if isinstance(alloc, mybir.MemoryLocationSet) and \
                alloc.memorylocations[0].name == "w":
            return tuple(alloc.tensor_shape) == (128, H)
    return False


def run_sharded(decoder_output, encoder_outputs, W, trace=False, nc=None, **kw):
    if nc is None:
        nc = _get_nc()
    in_maps = shard_inputs(decoder_output, encoder_outputs, W,
                           wshard=nc_is_wshard(nc))
    res = run_bass_kernel_spmd(nc, in_maps, list(range(NCORES)), trace=trace, **kw)
    attn = np.concatenate([res.results[c]["attn"] for c in range(NCORES)], axis=0)
    return attn, res


def kernel(decoder_output, encoder_outputs, W, b=None, **_unused):
    # b (the Linear bias) shifts every energy of a batch equally -> cancels in
    # softmax; it is deliberately unused.
    attn, _ = run_sharded(decoder_output, encoder_outputs, W)
    return attn.reshape(B, T, 1).astype(np.float32)

